# revision 59
# baseline (speedup 1.0000x reference)
"""Trainium2 Bass kernel for banded (sliding-window) single-head attention.

Problem (hardcoded):
    x     [256, 256, 768] f32   (batch, tokens, dim)
    w_qkv [768, 192]      f32
    w_out [64, 768]       f32
    b_out [768]           f32
    y = (softmax(band_mask(q k^T / 8)) v) @ w_out + b_out,  band |i-j| < 32

Strategy: pure data parallel over batch (32 batches/core on 8 cores).

v2 design (single fp16 x plane, fp16 output, band-narrowed attention):
  - x is cast to fp16 on host and laid out pre-transposed+blocked
    ([ptile, chunk, 128 feat, 512 tok]) so the device loads it with a
    plain contiguous DMA (1KB packets) -- no xbar transpose.
  - QKV: qkT [128(q|k), 512] and vT [64, 512] via 6-chunk accumulation.
  - Per batch (256 tokens): keys chunk jc (128 keys) only attends a
    160-wide query window (jc=0 -> q[0:160), jc=1 -> q[96:256)), so
    scores/exp/mask run on [128, 320] per batch instead of [128, 512].
  - v natural via PE transposes of vT + ones column -> vaug [128, 4, 65];
    PV accumulates the overlapping query region in PSUM (4 matmuls/batch).
  - o row 64 = softmax sums; select-matmul transposes them to partitions,
    reciprocal gives per-token 1/s fused into the final PSUM->SBUF copies.
  - final [tok chunk, 768] = osc.T @ [w_out; b_out] (ones row applies the
    bias), output fp16, host casts to fp32.
"""

import numpy as np

import concourse.bass as bass
import concourse.mybir as mybir
import concourse.tile as tile
from concourse import bacc
from concourse import bass_utils

F32 = mybir.dt.float32
F16 = mybir.dt.float16

B, N, D, DH = 256, 256, 768, 64
SA = 32                       # band half-width: |i-j| < SA
NCORES = 8
BLOC = B // NCORES            # batches per core
TOK_FULL = BLOC * N           # tokens per core (8192)
PT = 512                      # tokens per pipeline tile (2 batches)
NC_CHUNKS = D // 128          # 6 contraction chunks
W = 160                       # per-key-chunk query window width


def build_body(tc, x_blk, w_qkv, w_out, b_out, y, tok, ctx, dbg=None):
    nc = tc.nc
    npt = tok // PT
    nbatch_pt = PT // N       # batches per ptile (2)

    const = ctx.enter_context(tc.tile_pool(name="const", bufs=1))
    xp_pool = ctx.enter_context(tc.tile_pool(name="xp", bufs=4))
    qkT_pool = ctx.enter_context(tc.tile_pool(name="qkT", bufs=6))
    vT_pool = ctx.enter_context(tc.tile_pool(name="vT", bufs=2))
    vaug_pool = ctx.enter_context(tc.tile_pool(name="vaug", bufs=4))
    pexp_pool = ctx.enter_context(tc.tile_pool(name="pexp", bufs=3))
    osc_pool = ctx.enter_context(tc.tile_pool(name="osc", bufs=3))
    rcol_pool = ctx.enter_context(tc.tile_pool(name="rcol", bufs=3))
    y_pool = ctx.enter_context(tc.tile_pool(name="ysb", bufs=2))

    # PSUM: 8 banks exactly:
    #   proj 2x2KB, mm 3x(1.25KB->1 bank; vt/rcol/sc_a/sc_b cycle),
    #   o 1x2KB, f 2x1.5KB
    ps_proj = ctx.enter_context(tc.tile_pool(name="psproj", bufs=2, space="PSUM"))
    ps_mm = ctx.enter_context(tc.tile_pool(name="psmm", bufs=3, space="PSUM"))
    ps_o = ctx.enter_context(tc.tile_pool(name="pso", bufs=1, space="PSUM"))
    ps_f = ctx.enter_context(tc.tile_pool(name="psf", bufs=2, space="PSUM"))

    # ---- constants ----
    # w_qkv rearranged so chunk c holds rows [c*128, (c+1)*128)
    wq_sb = const.tile([128, NC_CHUNKS, 192], F16)
    nc.sync.dma_start(out=wq_sb[:], in_=w_qkv.rearrange("(c p) e -> p c e", p=128))

    # [w_out; b_out] as a 65-row augmented matrix
    waug = const.tile([65, D], F16)
    nc.sync.dma_start(out=waug[0:64, :], in_=w_out[:, :])
    nc.sync.dma_start(out=waug[64:65, :], in_=b_out.unsqueeze(0))

    e65 = const.tile([65, 1], F16)
    nc.vector.memset(e65[:], 0.0)
    nc.vector.memset(e65[64:65, :], 1.0)

    # band masks for the two windowed key chunks:
    #   jc=0: q = w, k = j      -> keep |w - j| <= 31
    #   jc=1: q = 96+w, k = 128+j -> keep 1 <= w - j <= 63
    maskt_f32 = const.tile([128, 2, W], F32)
    nc.gpsimd.memset(maskt_f32[:], 1.0)
    # jc=0: keep (31 + w - j >= 0) and (31 + j - w >= 0)
    nc.gpsimd.affine_select(
        out=maskt_f32[:, 0, :], in_=maskt_f32[:, 0, :],
        compare_op=mybir.AluOpType.is_ge, fill=0.0,
        base=SA - 1, channel_multiplier=-1, pattern=[[1, W]],
    )
    nc.gpsimd.affine_select(
        out=maskt_f32[:, 0, :], in_=maskt_f32[:, 0, :],
        compare_op=mybir.AluOpType.is_ge, fill=0.0,
        base=SA - 1, channel_multiplier=1, pattern=[[-1, W]],
    )
    # jc=1: keep (w - j - 1 >= 0) and (63 + j - w >= 0)
    nc.gpsimd.affine_select(
        out=maskt_f32[:, 1, :], in_=maskt_f32[:, 1, :],
        compare_op=mybir.AluOpType.is_ge, fill=0.0,
        base=-1, channel_multiplier=-1, pattern=[[1, W]],
    )
    nc.gpsimd.affine_select(
        out=maskt_f32[:, 1, :], in_=maskt_f32[:, 1, :],
        compare_op=mybir.AluOpType.is_ge, fill=0.0,
        base=2 * SA - 1, channel_multiplier=1, pattern=[[-1, W]],
    )
    maskt = const.tile([128, 2, W], F16)
    nc.scalar.copy(maskt[:], maskt_f32[:])

    # identity for PE transposes of vT slices
    ident_f32 = const.tile([64, 64], F32)
    nc.gpsimd.memset(ident_f32[:], 0.0)
    nc.gpsimd.affine_select(
        out=ident_f32[:], in_=ident_f32[:],
        compare_op=mybir.AluOpType.not_equal, fill=1.0,
        base=0, channel_multiplier=1, pattern=[[-1, 64]],
    )
    ident = const.tile([64, 64], F16)
    nc.scalar.copy(ident[:], ident_f32[:])

    def final_stage(pt, osc, rcol):
        """Output projection of ptile pt; software-pipelined two ptiles late
        with rcol precomputed, so the f matmuls and y copies are a pure
        MM->copy ping-pong with no in-loop latency chain."""
        y_sb = y_pool.tile([128, 4, D], F16, tag="ysb")
        for kc in range(4):
            lhsT = osc[:, kc * 128:(kc + 1) * 128]
            rc = rcol[:, kc:kc + 1]
            for half in range(2):
                f_ps = ps_f.tile([128, 384], F32, tag="f")
                nc.tensor.matmul(
                    f_ps[:], lhsT=lhsT,
                    rhs=waug[:, half * 384:(half + 1) * 384],
                    start=True, stop=True,
                )
                dst = y_sb[:, kc, half * 384:(half + 1) * 384]
                if half == 0:
                    nc.scalar.activation(
                        dst, f_ps[:],
                        mybir.ActivationFunctionType.Copy, scale=rc,
                    )
                else:
                    nc.vector.tensor_scalar_mul(dst, f_ps[:], rc)

        # partition-major DRAM layout: each partition's 6KB is one
        # contiguous row (host un-permutes); big packets, few descriptors
        nc.scalar.dma_start(
            out=y[pt * 128:(pt + 1) * 128, :].rearrange(
                "p (kc d) -> p kc d", kc=4),
            in_=y_sb[:],
        )

    def attn_back(vaug, pexp):
        """PV + PSUM->SBUF copy of the attention output (one ptile late),
        plus the softmax-sum transpose (select matmuls) and reciprocal so
        rcol is ready a full iteration before the final stage needs it."""
        o_ps = ps_o.tile([65, PT], F32, tag="o")
        for bb in range(nbatch_pt):
            q0 = bb * N
            # q[0:160] from key chunk jc0; q[96:256] from jc1 with the
            # overlap [96:160] accumulated in PSUM (per-element has_written)
            nc.tensor.matmul(
                o_ps[:, q0 + 0: q0 + 160], lhsT=vaug[:, bb * 2 + 0, :],
                rhs=pexp[:, bb, 0, :],
                start=(bb == 0), stop=True, skip_group_check=True,
            )
            nc.tensor.matmul(
                o_ps[:, q0 + 96: q0 + 256], lhsT=vaug[:, bb * 2 + 1, :],
                rhs=pexp[:, bb, 1, :],
                start=False, stop=True, skip_group_check=True,
            )
        osc = osc_pool.tile([65, PT], F16, tag="osc")
        nc.vector.tensor_copy(osc[:], o_ps[:])

        rcol_ps = ps_mm.tile([128, 4], F32, tag="mm")
        for kc in range(4):
            nc.tensor.matmul(
                rcol_ps[:, kc:kc + 1],
                lhsT=osc[:, kc * 128:(kc + 1) * 128], rhs=e65[:],
                start=True, stop=True,
            )
        rcol = rcol_pool.tile([128, 4], F32, tag="rcol")
        nc.vector.reciprocal(rcol[:], rcol_ps[:])
        return osc, rcol

    # ---- main pipeline over ptiles of PT tokens (2-stage skew) ----
    prev_attn = None          # (vaug, pexp) of pt-1 awaiting PV
    prev_osc = None           # (pt, osc) of pt-2 awaiting final stage
    def scores_stage(qT, kT, dbg_tap=False):
        """Windowed scores + exp + mask; qT/kT are a full iteration old so
        the sc matmuls never wait on the projection copies."""
        sc_list = []
        for bb in range(nbatch_pt):
            t0 = bb * N
            sc_ps = ps_mm.tile([128, 2, W], F32, tag="mm")
            for jc in range(2):
                nc.tensor.matmul(
                    sc_ps[:, jc, :],
                    lhsT=kT[:, t0 + jc * 128: t0 + (jc + 1) * 128],
                    rhs=qT[:, t0 + jc * 96: t0 + jc * 96 + W],
                    start=True, stop=True,
                )
            sc_list.append(sc_ps)
        pexp = pexp_pool.tile([128, nbatch_pt, 2, W], F16, tag="pexp")
        for bb in range(nbatch_pt):
            nc.scalar.activation(
                pexp[:, bb, :, :], sc_list[bb][:],
                mybir.ActivationFunctionType.Exp, scale=float(DH) ** -0.5,
            )
        nc.gpsimd.tensor_mul(pexp[:, 0, :, :], pexp[:, 0, :, :], maskt[:])
        nc.gpsimd.tensor_mul(pexp[:, 1, :, :], pexp[:, 1, :, :], maskt[:])
        if dbg_tap:
            nc.sync.dma_start(out=dbg["pexp"][:, :], in_=pexp[:, 0, :, :])
        return pexp

    # ---- skew ladder: proj(pt) | scores(pt-1) | PV(pt-2) | final(pt-3),
    # so every PE op's inputs are at least one iteration old ----
    qk_stage = None           # (qT, kT) of pt-1 awaiting scores
    attn_stage = None         # (vaug, pexp) of pt-2 awaiting PV
    fin_stage = None          # (pt, osc, rcol) of pt-3 awaiting final
    for pt in range(npt):
        xp = xp_pool.tile([128, NC_CHUNKS, PT], F16, tag="xp")
        nc.sync.dma_start(
            out=xp[:],
            in_=x_blk[pt * 128:(pt + 1) * 128, :]
            .rearrange("p (c t) -> p c t", c=NC_CHUNKS),
        )

        # vT first: its PSUM->SBUF copy is the transposes' dependency
        v_ps = ps_proj.tile([64, PT], F32, tag="proj")
        for c in range(NC_CHUNKS):
            nc.tensor.matmul(
                v_ps[:], lhsT=wq_sb[:, c, 128:192], rhs=xp[:, c, :],
                start=(c == 0), stop=(c == NC_CHUNKS - 1),
            )
        vT = vT_pool.tile([64, PT], F16, tag="vT")
        nc.scalar.copy(vT[:], v_ps[:])

        # qkT: [128(e = q|k), PT] = sum_c W_qk[c].T @ x[c]
        qk_ps = ps_proj.tile([128, PT], F32, tag="proj")
        for c in range(NC_CHUNKS):
            nc.tensor.matmul(
                qk_ps[:], lhsT=wq_sb[:, c, 0:128], rhs=xp[:, c, :],
                start=(c == 0), stop=(c == NC_CHUNKS - 1),
            )
        qT = qkT_pool.tile([64, PT], F16, tag="qT")
        nc.scalar.copy(qT[:], qk_ps[0:64, :])
        kT = qkT_pool.tile([64, PT], F16, tag="kT")
        nc.vector.tensor_copy(kT[:], qk_ps[64:128, :])

        if dbg is not None and pt == 0:
            nc.sync.dma_start(out=dbg["qT"][:, :], in_=qT[:])
            nc.sync.dma_start(out=dbg["kT"][:, :], in_=kT[:])
            nc.sync.dma_start(out=dbg["vT"][:, :], in_=vT[:])

        # v natural (+ ones col) via PE transposes
        vt_ps = ps_mm.tile([128, 4, 64], F16, tag="mm")
        for kc in range(4):
            nc.tensor.transpose(
                vt_ps[:, kc, :], vT[:, kc * 128:(kc + 1) * 128], ident[:],
            )
        vaug = vaug_pool.tile([128, 4, 65], F16, tag="vaug")
        nc.vector.tensor_copy(vaug[:, :, 0:64], vt_ps[:])
        nc.gpsimd.memset(vaug[:, :, 64:65], 1.0)
        if dbg is not None and pt == 0:
            nc.sync.dma_start(out=dbg["vaug"][:, :], in_=vaug[:])

        # skewed back-stages, all with iteration-old inputs; final before
        # scores so the y copies precede exp on ACT (exp has a full
        # iteration of slack, the y copies gate the f-bank ping-pong)
        if attn_stage is not None:
            osc, rcol = attn_back(*attn_stage[1:])
            if dbg is not None and attn_stage[0] == 0:
                nc.sync.dma_start(out=dbg["osc"][:, :], in_=osc[:, 0:N])
        if fin_stage is not None:
            final_stage(*fin_stage)
        pexp = scores_stage(*qk_stage[1:]) if qk_stage is not None else None
        if attn_stage is not None:
            fin_stage = (attn_stage[0], osc, rcol)
        if qk_stage is not None:
            attn_stage = (qk_stage[0], prev_vaug, pexp)
        qk_stage = (pt, qT, kT)
        prev_vaug = vaug

    # drain the ladder
    pexp = scores_stage(*qk_stage[1:])
    osc, rcol = attn_back(*attn_stage[1:])
    final_stage(*fin_stage)
    fin_stage = (attn_stage[0], osc, rcol)
    osc, rcol = attn_back(prev_vaug, pexp)
    final_stage(*fin_stage)
    final_stage(qk_stage[0], osc, rcol)


def build_nc(tok=TOK_FULL, debug_taps=False):
    nc = bacc.Bacc("TRN2", target_bir_lowering=False, debug=False)
    # x fp16, host-blocked partition-major: row = pt*128 + p holds that
    # partition's full 6KB [c, t] slice contiguously
    x_blk = nc.dram_tensor("x_blk", [tok // PT * 128, NC_CHUNKS * PT], F16,
                           kind="ExternalInput").ap()
    w_qkv = nc.dram_tensor("w_qkv", [D, 3 * DH], F16, kind="ExternalInput").ap()
    w_out = nc.dram_tensor("w_out", [DH, D], F16, kind="ExternalInput").ap()
    b_out = nc.dram_tensor("b_out", [D], F16, kind="ExternalInput").ap()
    # y partition-major per ptile (host un-permutes): row pt*128+p = [kc, d]
    y = nc.dram_tensor("y", [tok // PT * 128, 4 * D], F16,
                       kind="ExternalOutput").ap()

    dbg = None
    if debug_taps:
        dbg = {
            "qT": nc.dram_tensor("dbg_qT", [64, PT], F16,
                                 kind="ExternalOutput").ap(),
            "kT": nc.dram_tensor("dbg_kT", [64, PT], F16,
                                 kind="ExternalOutput").ap(),
            "vT": nc.dram_tensor("dbg_vT", [64, PT], F16,
                                 kind="ExternalOutput").ap(),
            "pexp": nc.dram_tensor("dbg_pexp", [128, 2 * W], F16,
                                   kind="ExternalOutput").ap(),
            "osc": nc.dram_tensor("dbg_osc", [65, N], F16,
                                  kind="ExternalOutput").ap(),
            "vaug": nc.dram_tensor("dbg_vaug", [128, 4 * 65], F16,
                                   kind="ExternalOutput").ap(),
        }

    from contextlib import ExitStack
    with tile.TileContext(nc) as tc:
        with ExitStack() as ctx:
            build_body(tc, x_blk, w_qkv, w_out, b_out, y, tok, ctx, dbg=dbg)
    nc.compile()
    return nc


def make_in_maps(x, w_qkv, w_out, b_out):
    in_maps = []
    w_qkv16 = np.asarray(w_qkv, dtype=np.float16)
    w_out16 = np.asarray(w_out, dtype=np.float16)
    b_out16 = np.asarray(b_out, dtype=np.float16)
    npt = TOK_FULL // PT
    for c in range(NCORES):
        xc = np.asarray(x)[c * BLOC:(c + 1) * BLOC].reshape(TOK_FULL, D)
        x16 = xc.astype(np.float16)
        # [tok, D] -> [npt, p, c, t]: per-partition rows are contiguous 6KB
        blk = np.ascontiguousarray(
            x16.reshape(npt, PT, NC_CHUNKS, 128).transpose(0, 3, 2, 1)
        ).reshape(npt * 128, NC_CHUNKS * PT)
        in_maps.append({
            "x_blk": blk,
            "w_qkv": w_qkv16, "w_out": w_out16, "b_out": b_out16,
        })
    return in_maps


_NC_CACHE = {}


def run(x, w_qkv, w_out, b_out, trace=False, **trace_kwargs):
    if "nc" not in _NC_CACHE:
        _NC_CACHE["nc"] = build_nc()
    nc = _NC_CACHE["nc"]
    in_maps = make_in_maps(x, w_qkv, w_out, b_out)
    res = bass_utils.run_bass_kernel_spmd(
        nc, in_maps, core_ids=list(range(NCORES)), trace=trace, **trace_kwargs
    )
    npt = TOK_FULL // PT
    y = np.concatenate(
        [
            res.results[c]["y"]
            .reshape(npt, 128, 4, D)
            .transpose(0, 2, 1, 3)
            .reshape(BLOC, N, D)
            for c in range(NCORES)
        ],
        axis=0,
    )
    return y.astype(np.float32), res


def kernel(x, w_qkv, w_out, b_out):
    y, _ = run(np.asarray(x), np.asarray(w_qkv), np.asarray(w_out),
               np.asarray(b_out))
    return y


# revision 60
# speedup vs baseline: 1.0250x; 1.0250x over previous
"""Trainium2 Bass kernel for banded (sliding-window) single-head attention.

Problem (hardcoded):
    x     [256, 256, 768] f32   (batch, tokens, dim)
    w_qkv [768, 192]      f32
    w_out [64, 768]       f32
    b_out [768]           f32
    y = (softmax(band_mask(q k^T / 8)) v) @ w_out + b_out,  band |i-j| < 32

Strategy: pure data parallel over batch (32 batches/core on 8 cores).

v2 design (single fp16 x plane, fp16 output, band-narrowed attention):
  - x is cast to fp16 on host and laid out pre-transposed+blocked
    ([ptile, chunk, 128 feat, 512 tok]) so the device loads it with a
    plain contiguous DMA (1KB packets) -- no xbar transpose.
  - QKV: qkT [128(q|k), 512] and vT [64, 512] via 6-chunk accumulation.
  - Per batch (256 tokens): keys chunk jc (128 keys) only attends a
    160-wide query window (jc=0 -> q[0:160), jc=1 -> q[96:256)), so
    scores/exp/mask run on [128, 320] per batch instead of [128, 512].
  - v natural via PE transposes of vT + ones column -> vaug [128, 4, 65];
    PV accumulates the overlapping query region in PSUM (4 matmuls/batch).
  - o row 64 = softmax sums; select-matmul transposes them to partitions,
    reciprocal gives per-token 1/s fused into the final PSUM->SBUF copies.
  - final [tok chunk, 768] = osc.T @ [w_out; b_out] (ones row applies the
    bias), output fp16, host casts to fp32.
"""

import numpy as np

import concourse.bass as bass
import concourse.mybir as mybir
import concourse.tile as tile
from concourse import bacc
from concourse import bass_utils

F32 = mybir.dt.float32
F16 = mybir.dt.float16

B, N, D, DH = 256, 256, 768, 64
SA = 32                       # band half-width: |i-j| < SA
NCORES = 8
BLOC = B // NCORES            # batches per core
TOK_FULL = BLOC * N           # tokens per core (8192)
PT = 512                      # tokens per pipeline tile (2 batches)
NC_CHUNKS = D // 128          # 6 contraction chunks
W = 160                       # per-key-chunk query window width


def build_body(tc, x_blk, w_qkv, w_out, b_out, y, tok, ctx, dbg=None):
    nc = tc.nc
    npt = tok // PT
    nbatch_pt = PT // N       # batches per ptile (2)

    const = ctx.enter_context(tc.tile_pool(name="const", bufs=1))
    xp_pool = ctx.enter_context(tc.tile_pool(name="xp", bufs=4))
    qkT_pool = ctx.enter_context(tc.tile_pool(name="qkT", bufs=6))
    vT_pool = ctx.enter_context(tc.tile_pool(name="vT", bufs=2))
    vaug_pool = ctx.enter_context(tc.tile_pool(name="vaug", bufs=4))
    pexp_pool = ctx.enter_context(tc.tile_pool(name="pexp", bufs=3))
    osc_pool = ctx.enter_context(tc.tile_pool(name="osc", bufs=3))
    rcol_pool = ctx.enter_context(tc.tile_pool(name="rcol", bufs=3))
    y_pool = ctx.enter_context(tc.tile_pool(name="ysb", bufs=2))

    # PSUM: 8 banks exactly:
    #   proj 2x2KB, mm 3x(1.25KB->1 bank; vt/rcol/sc_a/sc_b cycle),
    #   o 1x2KB, f 2x1.5KB
    ps_proj = ctx.enter_context(tc.tile_pool(name="psproj", bufs=2, space="PSUM"))
    ps_mm = ctx.enter_context(tc.tile_pool(name="psmm", bufs=3, space="PSUM"))
    ps_o = ctx.enter_context(tc.tile_pool(name="pso", bufs=1, space="PSUM"))
    ps_f = ctx.enter_context(tc.tile_pool(name="psf", bufs=2, space="PSUM"))

    # ---- constants ----
    # w_qkv rearranged so chunk c holds rows [c*128, (c+1)*128)
    wq_sb = const.tile([128, NC_CHUNKS, 192], F16)
    nc.sync.dma_start(out=wq_sb[:], in_=w_qkv.rearrange("(c p) e -> p c e", p=128))

    # [w_out; b_out] as a 65-row augmented matrix
    waug = const.tile([65, D], F16)
    nc.sync.dma_start(out=waug[0:64, :], in_=w_out[:, :])
    nc.sync.dma_start(out=waug[64:65, :], in_=b_out.unsqueeze(0))

    e65 = const.tile([65, 1], F16)
    nc.vector.memset(e65[:], 0.0)
    nc.vector.memset(e65[64:65, :], 1.0)

    # band masks for the two windowed key chunks:
    #   jc=0: q = w, k = j      -> keep |w - j| <= 31
    #   jc=1: q = 96+w, k = 128+j -> keep 1 <= w - j <= 63
    maskt_f32 = const.tile([128, 2, W], F32)
    nc.gpsimd.memset(maskt_f32[:], 1.0)
    # jc=0: keep (31 + w - j >= 0) and (31 + j - w >= 0)
    nc.gpsimd.affine_select(
        out=maskt_f32[:, 0, :], in_=maskt_f32[:, 0, :],
        compare_op=mybir.AluOpType.is_ge, fill=0.0,
        base=SA - 1, channel_multiplier=-1, pattern=[[1, W]],
    )
    nc.gpsimd.affine_select(
        out=maskt_f32[:, 0, :], in_=maskt_f32[:, 0, :],
        compare_op=mybir.AluOpType.is_ge, fill=0.0,
        base=SA - 1, channel_multiplier=1, pattern=[[-1, W]],
    )
    # jc=1: keep (w - j - 1 >= 0) and (63 + j - w >= 0)
    nc.gpsimd.affine_select(
        out=maskt_f32[:, 1, :], in_=maskt_f32[:, 1, :],
        compare_op=mybir.AluOpType.is_ge, fill=0.0,
        base=-1, channel_multiplier=-1, pattern=[[1, W]],
    )
    nc.gpsimd.affine_select(
        out=maskt_f32[:, 1, :], in_=maskt_f32[:, 1, :],
        compare_op=mybir.AluOpType.is_ge, fill=0.0,
        base=2 * SA - 1, channel_multiplier=1, pattern=[[-1, W]],
    )
    maskt = const.tile([128, 2, W], F16)
    nc.scalar.copy(maskt[:], maskt_f32[:])

    # identity for PE transposes of vT slices
    ident_f32 = const.tile([64, 64], F32)
    nc.gpsimd.memset(ident_f32[:], 0.0)
    nc.gpsimd.affine_select(
        out=ident_f32[:], in_=ident_f32[:],
        compare_op=mybir.AluOpType.not_equal, fill=1.0,
        base=0, channel_multiplier=1, pattern=[[-1, 64]],
    )
    ident = const.tile([64, 64], F16)
    nc.scalar.copy(ident[:], ident_f32[:])

    def final_stage(pt, osc, rcol):
        """Output projection of ptile pt; software-pipelined two ptiles late
        with rcol precomputed, so the f matmuls and y copies are a pure
        MM->copy ping-pong with no in-loop latency chain."""
        y_sb = y_pool.tile([128, 4, D], F16, tag="ysb")
        for kc in range(4):
            lhsT = osc[:, kc * 128:(kc + 1) * 128]
            rc = rcol[:, kc:kc + 1]
            for half in range(2):
                f_ps = ps_f.tile([128, 384], F32, tag="f")
                nc.tensor.matmul(
                    f_ps[:], lhsT=lhsT,
                    rhs=waug[:, half * 384:(half + 1) * 384],
                    start=True, stop=True,
                )
                dst = y_sb[:, kc, half * 384:(half + 1) * 384]
                if half == 0:
                    nc.scalar.activation(
                        dst, f_ps[:],
                        mybir.ActivationFunctionType.Copy, scale=rc,
                    )
                else:
                    nc.vector.tensor_scalar_mul(dst, f_ps[:], rc)

        # partition-major DRAM layout: each partition's 6KB is one
        # contiguous row (host un-permutes); big packets, few descriptors
        nc.scalar.dma_start(
            out=y[pt * 128:(pt + 1) * 128, :].rearrange(
                "p (kc d) -> p kc d", kc=4),
            in_=y_sb[:],
        )

    def attn_back(vaug, pexp):
        """PV + PSUM->SBUF copy of the attention output (one ptile late),
        plus the softmax-sum transpose (select matmuls) and reciprocal so
        rcol is ready a full iteration before the final stage needs it."""
        o_ps = ps_o.tile([65, PT], F32, tag="o")
        for bb in range(nbatch_pt):
            q0 = bb * N
            # q[0:160] from key chunk jc0; q[96:256] from jc1 with the
            # overlap [96:160] accumulated in PSUM (per-element has_written)
            nc.tensor.matmul(
                o_ps[:, q0 + 0: q0 + 160], lhsT=vaug[:, bb * 2 + 0, :],
                rhs=pexp[:, bb, 0, :],
                start=(bb == 0), stop=True, skip_group_check=True,
            )
            nc.tensor.matmul(
                o_ps[:, q0 + 96: q0 + 256], lhsT=vaug[:, bb * 2 + 1, :],
                rhs=pexp[:, bb, 1, :],
                start=False, stop=True, skip_group_check=True,
            )
        osc = osc_pool.tile([65, PT], F16, tag="osc")
        nc.vector.tensor_copy(osc[:], o_ps[:])

        rcol_ps = ps_mm.tile([128, 4], F32, tag="mm")
        for kc in range(4):
            nc.tensor.matmul(
                rcol_ps[:, kc:kc + 1],
                lhsT=osc[:, kc * 128:(kc + 1) * 128], rhs=e65[:],
                start=True, stop=True,
            )
        rcol = rcol_pool.tile([128, 4], F32, tag="rcol")
        nc.vector.reciprocal(rcol[:], rcol_ps[:])
        return osc, rcol

    # ---- main pipeline over ptiles of PT tokens (2-stage skew) ----
    prev_attn = None          # (vaug, pexp) of pt-1 awaiting PV
    prev_osc = None           # (pt, osc) of pt-2 awaiting final stage
    def scores_stage(qT, kT, dbg_tap=False):
        """Windowed scores + exp + mask; qT/kT are a full iteration old so
        the sc matmuls never wait on the projection copies."""
        sc_list = []
        for bb in range(nbatch_pt):
            t0 = bb * N
            sc_ps = ps_mm.tile([128, 2, W], F32, tag="mm")
            for jc in range(2):
                nc.tensor.matmul(
                    sc_ps[:, jc, :],
                    lhsT=kT[:, t0 + jc * 128: t0 + (jc + 1) * 128],
                    rhs=qT[:, t0 + jc * 96: t0 + jc * 96 + W],
                    start=True, stop=True,
                )
            sc_list.append(sc_ps)
        pexp = pexp_pool.tile([128, nbatch_pt, 2, W], F16, tag="pexp")
        for bb in range(nbatch_pt):
            nc.scalar.activation(
                pexp[:, bb, :, :], sc_list[bb][:],
                mybir.ActivationFunctionType.Exp, scale=float(DH) ** -0.5,
            )
        nc.gpsimd.tensor_mul(pexp[:, 0, :, :], pexp[:, 0, :, :], maskt[:])
        nc.gpsimd.tensor_mul(pexp[:, 1, :, :], pexp[:, 1, :, :], maskt[:])
        if dbg_tap:
            nc.sync.dma_start(out=dbg["pexp"][:, :], in_=pexp[:, 0, :, :])
        return pexp

    # ---- skew ladder: proj(pt) | scores(pt-1) | PV(pt-2) | final(pt-3),
    # so every PE op's inputs are at least one iteration old ----
    qk_stage = None           # (qT, kT) of pt-1 awaiting scores
    attn_stage = None         # (vaug, pexp) of pt-2 awaiting PV
    fin_stage = None          # (pt, osc, rcol) of pt-3 awaiting final
    for pt in range(npt):
        xp = xp_pool.tile([128, NC_CHUNKS, PT], F16, tag="xp")
        nc.sync.dma_start(
            out=xp[:],
            in_=x_blk[pt * 128:(pt + 1) * 128, :]
            .rearrange("p (c t) -> p c t", c=NC_CHUNKS),
        )

        # vT first: its PSUM->SBUF copy is the transposes' dependency
        v_ps = ps_proj.tile([64, PT], F32, tag="proj")
        for c in range(NC_CHUNKS):
            nc.tensor.matmul(
                v_ps[:], lhsT=wq_sb[:, c, 128:192], rhs=xp[:, c, :],
                start=(c == 0), stop=(c == NC_CHUNKS - 1),
            )
        vT = vT_pool.tile([64, PT], F16, tag="vT")
        nc.scalar.copy(vT[:], v_ps[:])

        # qkT: [128(e = q|k), PT] = sum_c W_qk[c].T @ x[c]
        qk_ps = ps_proj.tile([128, PT], F32, tag="proj")
        for c in range(NC_CHUNKS):
            nc.tensor.matmul(
                qk_ps[:], lhsT=wq_sb[:, c, 0:128], rhs=xp[:, c, :],
                start=(c == 0), stop=(c == NC_CHUNKS - 1),
            )
        qT = qkT_pool.tile([64, PT], F16, tag="qT")
        nc.scalar.copy(qT[:], qk_ps[0:64, :])
        kT = qkT_pool.tile([64, PT], F16, tag="kT")
        nc.vector.tensor_copy(kT[:], qk_ps[64:128, :])

        if dbg is not None and pt == 0:
            nc.sync.dma_start(out=dbg["qT"][:, :], in_=qT[:])
            nc.sync.dma_start(out=dbg["kT"][:, :], in_=kT[:])
            nc.sync.dma_start(out=dbg["vT"][:, :], in_=vT[:])

        # v natural (+ ones col) via PE transposes
        vt_ps = ps_mm.tile([128, 4, 64], F16, tag="mm")
        for kc in range(4):
            nc.tensor.transpose(
                vt_ps[:, kc, :], vT[:, kc * 128:(kc + 1) * 128], ident[:],
            )
        vaug = vaug_pool.tile([128, 4, 65], F16, tag="vaug")
        nc.vector.tensor_copy(vaug[:, :, 0:64], vt_ps[:])
        nc.gpsimd.memset(vaug[:, :, 64:65], 1.0)
        if dbg is not None and pt == 0:
            nc.sync.dma_start(out=dbg["vaug"][:, :], in_=vaug[:])

        # skewed back-stages, all with iteration-old inputs
        pexp = scores_stage(*qk_stage[1:]) if qk_stage is not None else None
        if attn_stage is not None:
            osc, rcol = attn_back(*attn_stage[1:])
            if dbg is not None and attn_stage[0] == 0:
                nc.sync.dma_start(out=dbg["osc"][:, :], in_=osc[:, 0:N])
        if fin_stage is not None:
            final_stage(*fin_stage)
        if attn_stage is not None:
            fin_stage = (attn_stage[0], osc, rcol)
        if qk_stage is not None:
            attn_stage = (qk_stage[0], prev_vaug, pexp)
        qk_stage = (pt, qT, kT)
        prev_vaug = vaug

    # drain the ladder
    pexp = scores_stage(*qk_stage[1:])
    osc, rcol = attn_back(*attn_stage[1:])
    final_stage(*fin_stage)
    fin_stage = (attn_stage[0], osc, rcol)
    osc, rcol = attn_back(prev_vaug, pexp)
    final_stage(*fin_stage)
    final_stage(qk_stage[0], osc, rcol)


def build_nc(tok=TOK_FULL, debug_taps=False):
    nc = bacc.Bacc("TRN2", target_bir_lowering=False, debug=False)
    # x fp16, host-blocked partition-major: row = pt*128 + p holds that
    # partition's full 6KB [c, t] slice contiguously
    x_blk = nc.dram_tensor("x_blk", [tok // PT * 128, NC_CHUNKS * PT], F16,
                           kind="ExternalInput").ap()
    w_qkv = nc.dram_tensor("w_qkv", [D, 3 * DH], F16, kind="ExternalInput").ap()
    w_out = nc.dram_tensor("w_out", [DH, D], F16, kind="ExternalInput").ap()
    b_out = nc.dram_tensor("b_out", [D], F16, kind="ExternalInput").ap()
    # y partition-major per ptile (host un-permutes): row pt*128+p = [kc, d]
    y = nc.dram_tensor("y", [tok // PT * 128, 4 * D], F16,
                       kind="ExternalOutput").ap()

    dbg = None
    if debug_taps:
        dbg = {
            "qT": nc.dram_tensor("dbg_qT", [64, PT], F16,
                                 kind="ExternalOutput").ap(),
            "kT": nc.dram_tensor("dbg_kT", [64, PT], F16,
                                 kind="ExternalOutput").ap(),
            "vT": nc.dram_tensor("dbg_vT", [64, PT], F16,
                                 kind="ExternalOutput").ap(),
            "pexp": nc.dram_tensor("dbg_pexp", [128, 2 * W], F16,
                                   kind="ExternalOutput").ap(),
            "osc": nc.dram_tensor("dbg_osc", [65, N], F16,
                                  kind="ExternalOutput").ap(),
            "vaug": nc.dram_tensor("dbg_vaug", [128, 4 * 65], F16,
                                   kind="ExternalOutput").ap(),
        }

    from contextlib import ExitStack
    with tile.TileContext(nc) as tc:
        with ExitStack() as ctx:
            build_body(tc, x_blk, w_qkv, w_out, b_out, y, tok, ctx, dbg=dbg)
    nc.compile()
    return nc


def make_in_maps(x, w_qkv, w_out, b_out):
    in_maps = []
    w_qkv16 = np.asarray(w_qkv, dtype=np.float16)
    w_out16 = np.asarray(w_out, dtype=np.float16)
    b_out16 = np.asarray(b_out, dtype=np.float16)
    npt = TOK_FULL // PT
    for c in range(NCORES):
        xc = np.asarray(x)[c * BLOC:(c + 1) * BLOC].reshape(TOK_FULL, D)
        x16 = xc.astype(np.float16)
        # [tok, D] -> [npt, p, c, t]: per-partition rows are contiguous 6KB
        blk = np.ascontiguousarray(
            x16.reshape(npt, PT, NC_CHUNKS, 128).transpose(0, 3, 2, 1)
        ).reshape(npt * 128, NC_CHUNKS * PT)
        in_maps.append({
            "x_blk": blk,
            "w_qkv": w_qkv16, "w_out": w_out16, "b_out": b_out16,
        })
    return in_maps


_NC_CACHE = {}


def run(x, w_qkv, w_out, b_out, trace=False, **trace_kwargs):
    if "nc" not in _NC_CACHE:
        _NC_CACHE["nc"] = build_nc()
    nc = _NC_CACHE["nc"]
    in_maps = make_in_maps(x, w_qkv, w_out, b_out)
    res = bass_utils.run_bass_kernel_spmd(
        nc, in_maps, core_ids=list(range(NCORES)), trace=trace, **trace_kwargs
    )
    npt = TOK_FULL // PT
    y = np.concatenate(
        [
            res.results[c]["y"]
            .reshape(npt, 128, 4, D)
            .transpose(0, 2, 1, 3)
            .reshape(BLOC, N, D)
            for c in range(NCORES)
        ],
        axis=0,
    )
    return y.astype(np.float32), res


def kernel(x, w_qkv, w_out, b_out):
    y, _ = run(np.asarray(x), np.asarray(w_qkv), np.asarray(w_out),
               np.asarray(b_out))
    return y


# revision 61
# speedup vs baseline: 1.0292x; 1.0041x over previous
"""Trainium2 Bass kernel for banded (sliding-window) single-head attention.

Problem (hardcoded):
    x     [256, 256, 768] f32   (batch, tokens, dim)
    w_qkv [768, 192]      f32
    w_out [64, 768]       f32
    b_out [768]           f32
    y = (softmax(band_mask(q k^T / 8)) v) @ w_out + b_out,  band |i-j| < 32

Strategy: pure data parallel over batch (32 batches/core on 8 cores).

v2 design (single fp16 x plane, fp16 output, band-narrowed attention):
  - x is cast to fp16 on host and laid out pre-transposed+blocked
    ([ptile, chunk, 128 feat, 512 tok]) so the device loads it with a
    plain contiguous DMA (1KB packets) -- no xbar transpose.
  - QKV: qkT [128(q|k), 512] and vT [64, 512] via 6-chunk accumulation.
  - Per batch (256 tokens): keys chunk jc (128 keys) only attends a
    160-wide query window (jc=0 -> q[0:160), jc=1 -> q[96:256)), so
    scores/exp/mask run on [128, 320] per batch instead of [128, 512].
  - v natural via PE transposes of vT + ones column -> vaug [128, 4, 65];
    PV accumulates the overlapping query region in PSUM (4 matmuls/batch).
  - o row 64 = softmax sums; select-matmul transposes them to partitions,
    reciprocal gives per-token 1/s fused into the final PSUM->SBUF copies.
  - final [tok chunk, 768] = osc.T @ [w_out; b_out] (ones row applies the
    bias), output fp16, host casts to fp32.
"""

import numpy as np

import concourse.bass as bass
import concourse.mybir as mybir
import concourse.tile as tile
from concourse import bacc
from concourse import bass_utils

F32 = mybir.dt.float32
F16 = mybir.dt.float16

B, N, D, DH = 256, 256, 768, 64
SA = 32                       # band half-width: |i-j| < SA
NCORES = 8
BLOC = B // NCORES            # batches per core
TOK_FULL = BLOC * N           # tokens per core (8192)
PT = 512                      # tokens per pipeline tile (2 batches)
NC_CHUNKS = D // 128          # 6 contraction chunks
W = 160                       # per-key-chunk query window width


def build_body(tc, x_blk, w_qkv, w_out, b_out, y, tok, ctx, dbg=None):
    nc = tc.nc
    npt = tok // PT
    nbatch_pt = PT // N       # batches per ptile (2)

    const = ctx.enter_context(tc.tile_pool(name="const", bufs=1))
    xp_pool = ctx.enter_context(tc.tile_pool(name="xp", bufs=4))
    qkT_pool = ctx.enter_context(tc.tile_pool(name="qkT", bufs=6))
    vT_pool = ctx.enter_context(tc.tile_pool(name="vT", bufs=2))
    vaug_pool = ctx.enter_context(tc.tile_pool(name="vaug", bufs=4))
    pexp_pool = ctx.enter_context(tc.tile_pool(name="pexp", bufs=4))
    osc_pool = ctx.enter_context(tc.tile_pool(name="osc", bufs=4))
    rcol_pool = ctx.enter_context(tc.tile_pool(name="rcol", bufs=3))
    y_pool = ctx.enter_context(tc.tile_pool(name="ysb", bufs=3))

    # PSUM: 8 banks exactly:
    #   proj 2x2KB, mm 3x(1.25KB->1 bank; vt/rcol/sc_a/sc_b cycle),
    #   o 1x2KB, f 2x1.5KB
    ps_proj = ctx.enter_context(tc.tile_pool(name="psproj", bufs=2, space="PSUM"))
    ps_mm = ctx.enter_context(tc.tile_pool(name="psmm", bufs=3, space="PSUM"))
    ps_o = ctx.enter_context(tc.tile_pool(name="pso", bufs=1, space="PSUM"))
    ps_f = ctx.enter_context(tc.tile_pool(name="psf", bufs=2, space="PSUM"))

    # ---- constants ----
    # w_qkv rearranged so chunk c holds rows [c*128, (c+1)*128)
    wq_sb = const.tile([128, NC_CHUNKS, 192], F16)
    nc.sync.dma_start(out=wq_sb[:], in_=w_qkv.rearrange("(c p) e -> p c e", p=128))

    # [w_out; b_out] as a 65-row augmented matrix
    waug = const.tile([65, D], F16)
    nc.sync.dma_start(out=waug[0:64, :], in_=w_out[:, :])
    nc.sync.dma_start(out=waug[64:65, :], in_=b_out.unsqueeze(0))

    e65 = const.tile([65, 1], F16)
    nc.vector.memset(e65[:], 0.0)
    nc.vector.memset(e65[64:65, :], 1.0)

    # band masks for the two windowed key chunks:
    #   jc=0: q = w, k = j      -> keep |w - j| <= 31
    #   jc=1: q = 96+w, k = 128+j -> keep 1 <= w - j <= 63
    maskt_f32 = const.tile([128, 2, W], F32)
    nc.gpsimd.memset(maskt_f32[:], 1.0)
    # jc=0: keep (31 + w - j >= 0) and (31 + j - w >= 0)
    nc.gpsimd.affine_select(
        out=maskt_f32[:, 0, :], in_=maskt_f32[:, 0, :],
        compare_op=mybir.AluOpType.is_ge, fill=0.0,
        base=SA - 1, channel_multiplier=-1, pattern=[[1, W]],
    )
    nc.gpsimd.affine_select(
        out=maskt_f32[:, 0, :], in_=maskt_f32[:, 0, :],
        compare_op=mybir.AluOpType.is_ge, fill=0.0,
        base=SA - 1, channel_multiplier=1, pattern=[[-1, W]],
    )
    # jc=1: keep (w - j - 1 >= 0) and (63 + j - w >= 0)
    nc.gpsimd.affine_select(
        out=maskt_f32[:, 1, :], in_=maskt_f32[:, 1, :],
        compare_op=mybir.AluOpType.is_ge, fill=0.0,
        base=-1, channel_multiplier=-1, pattern=[[1, W]],
    )
    nc.gpsimd.affine_select(
        out=maskt_f32[:, 1, :], in_=maskt_f32[:, 1, :],
        compare_op=mybir.AluOpType.is_ge, fill=0.0,
        base=2 * SA - 1, channel_multiplier=1, pattern=[[-1, W]],
    )
    maskt = const.tile([128, 2, W], F16)
    nc.scalar.copy(maskt[:], maskt_f32[:])

    # identity for PE transposes of vT slices
    ident_f32 = const.tile([64, 64], F32)
    nc.gpsimd.memset(ident_f32[:], 0.0)
    nc.gpsimd.affine_select(
        out=ident_f32[:], in_=ident_f32[:],
        compare_op=mybir.AluOpType.not_equal, fill=1.0,
        base=0, channel_multiplier=1, pattern=[[-1, 64]],
    )
    ident = const.tile([64, 64], F16)
    nc.scalar.copy(ident[:], ident_f32[:])

    def final_stage(pt, osc, rcol):
        """Output projection of ptile pt; software-pipelined two ptiles late
        with rcol precomputed, so the f matmuls and y copies are a pure
        MM->copy ping-pong with no in-loop latency chain."""
        y_sb = y_pool.tile([128, 4, D], F16, tag="ysb")
        for kc in range(4):
            lhsT = osc[:, kc * 128:(kc + 1) * 128]
            rc = rcol[:, kc:kc + 1]
            for half in range(2):
                f_ps = ps_f.tile([128, 384], F32, tag="f")
                nc.tensor.matmul(
                    f_ps[:], lhsT=lhsT,
                    rhs=waug[:, half * 384:(half + 1) * 384],
                    start=True, stop=True,
                )
                dst = y_sb[:, kc, half * 384:(half + 1) * 384]
                if half == 0:
                    nc.scalar.activation(
                        dst, f_ps[:],
                        mybir.ActivationFunctionType.Copy, scale=rc,
                    )
                else:
                    nc.vector.tensor_scalar_mul(dst, f_ps[:], rc)

        # partition-major DRAM layout: each partition's 6KB is one
        # contiguous row (host un-permutes); big packets, few descriptors
        nc.scalar.dma_start(
            out=y[pt * 128:(pt + 1) * 128, :].rearrange(
                "p (kc d) -> p kc d", kc=4),
            in_=y_sb[:],
        )

    def attn_back(vaug, pexp):
        """PV + PSUM->SBUF copy of the attention output (one ptile late),
        plus the softmax-sum transpose (select matmuls) and reciprocal so
        rcol is ready a full iteration before the final stage needs it."""
        o_ps = ps_o.tile([65, PT], F32, tag="o")
        for bb in range(nbatch_pt):
            q0 = bb * N
            # q[0:160] from key chunk jc0; q[96:256] from jc1 with the
            # overlap [96:160] accumulated in PSUM (per-element has_written)
            nc.tensor.matmul(
                o_ps[:, q0 + 0: q0 + 160], lhsT=vaug[:, bb * 2 + 0, :],
                rhs=pexp[:, bb, 0, :],
                start=(bb == 0), stop=True, skip_group_check=True,
            )
            nc.tensor.matmul(
                o_ps[:, q0 + 96: q0 + 256], lhsT=vaug[:, bb * 2 + 1, :],
                rhs=pexp[:, bb, 1, :],
                start=False, stop=True, skip_group_check=True,
            )
        osc = osc_pool.tile([65, PT], F16, tag="osc")
        nc.vector.tensor_copy(osc[:], o_ps[:])

        rcol_ps = ps_mm.tile([128, 4], F32, tag="mm")
        for kc in range(4):
            nc.tensor.matmul(
                rcol_ps[:, kc:kc + 1],
                lhsT=osc[:, kc * 128:(kc + 1) * 128], rhs=e65[:],
                start=True, stop=True,
            )
        rcol = rcol_pool.tile([128, 4], F32, tag="rcol")
        nc.vector.reciprocal(rcol[:], rcol_ps[:])
        return osc, rcol

    # ---- main pipeline over ptiles of PT tokens (2-stage skew) ----
    prev_attn = None          # (vaug, pexp) of pt-1 awaiting PV
    prev_osc = None           # (pt, osc) of pt-2 awaiting final stage
    def scores_stage(qT, kT, dbg_tap=False):
        """Windowed scores + exp + mask; qT/kT are a full iteration old so
        the sc matmuls never wait on the projection copies."""
        sc_list = []
        for bb in range(nbatch_pt):
            t0 = bb * N
            sc_ps = ps_mm.tile([128, 2, W], F32, tag="mm")
            for jc in range(2):
                nc.tensor.matmul(
                    sc_ps[:, jc, :],
                    lhsT=kT[:, t0 + jc * 128: t0 + (jc + 1) * 128],
                    rhs=qT[:, t0 + jc * 96: t0 + jc * 96 + W],
                    start=True, stop=True,
                )
            sc_list.append(sc_ps)
        pexp = pexp_pool.tile([128, nbatch_pt, 2, W], F16, tag="pexp")
        for bb in range(nbatch_pt):
            nc.scalar.activation(
                pexp[:, bb, :, :], sc_list[bb][:],
                mybir.ActivationFunctionType.Exp, scale=float(DH) ** -0.5,
            )
        nc.gpsimd.tensor_mul(pexp[:, 0, :, :], pexp[:, 0, :, :], maskt[:])
        nc.gpsimd.tensor_mul(pexp[:, 1, :, :], pexp[:, 1, :, :], maskt[:])
        if dbg_tap:
            nc.sync.dma_start(out=dbg["pexp"][:, :], in_=pexp[:, 0, :, :])
        return pexp

    # ---- skew ladder: proj(pt) | scores(pt-1) | PV(pt-2) | final(pt-3),
    # so every PE op's inputs are at least one iteration old ----
    qk_stage = None           # (qT, kT) of pt-1 awaiting scores
    attn_stage = None         # (vaug, pexp) of pt-2 awaiting PV
    fin_stage = None          # (pt, osc, rcol) of pt-3 awaiting final
    for pt in range(npt):
        xp = xp_pool.tile([128, NC_CHUNKS, PT], F16, tag="xp")
        nc.sync.dma_start(
            out=xp[:],
            in_=x_blk[pt * 128:(pt + 1) * 128, :]
            .rearrange("p (c t) -> p c t", c=NC_CHUNKS),
        )

        # vT first: its PSUM->SBUF copy is the transposes' dependency
        v_ps = ps_proj.tile([64, PT], F32, tag="proj")
        for c in range(NC_CHUNKS):
            nc.tensor.matmul(
                v_ps[:], lhsT=wq_sb[:, c, 128:192], rhs=xp[:, c, :],
                start=(c == 0), stop=(c == NC_CHUNKS - 1),
            )
        vT = vT_pool.tile([64, PT], F16, tag="vT")
        nc.scalar.copy(vT[:], v_ps[:])

        # qkT: [128(e = q|k), PT] = sum_c W_qk[c].T @ x[c]
        qk_ps = ps_proj.tile([128, PT], F32, tag="proj")
        for c in range(NC_CHUNKS):
            nc.tensor.matmul(
                qk_ps[:], lhsT=wq_sb[:, c, 0:128], rhs=xp[:, c, :],
                start=(c == 0), stop=(c == NC_CHUNKS - 1),
            )
        qT = qkT_pool.tile([64, PT], F16, tag="qT")
        nc.scalar.copy(qT[:], qk_ps[0:64, :])
        kT = qkT_pool.tile([64, PT], F16, tag="kT")
        nc.vector.tensor_copy(kT[:], qk_ps[64:128, :])

        if dbg is not None and pt == 0:
            nc.sync.dma_start(out=dbg["qT"][:, :], in_=qT[:])
            nc.sync.dma_start(out=dbg["kT"][:, :], in_=kT[:])
            nc.sync.dma_start(out=dbg["vT"][:, :], in_=vT[:])

        # v natural (+ ones col) via PE transposes
        vt_ps = ps_mm.tile([128, 4, 64], F16, tag="mm")
        for kc in range(4):
            nc.tensor.transpose(
                vt_ps[:, kc, :], vT[:, kc * 128:(kc + 1) * 128], ident[:],
            )
        vaug = vaug_pool.tile([128, 4, 65], F16, tag="vaug")
        nc.vector.tensor_copy(vaug[:, :, 0:64], vt_ps[:])
        nc.gpsimd.memset(vaug[:, :, 64:65], 1.0)
        if dbg is not None and pt == 0:
            nc.sync.dma_start(out=dbg["vaug"][:, :], in_=vaug[:])

        # skewed back-stages, all with iteration-old inputs
        pexp = scores_stage(*qk_stage[1:]) if qk_stage is not None else None
        if attn_stage is not None:
            osc, rcol = attn_back(*attn_stage[1:])
            if dbg is not None and attn_stage[0] == 0:
                nc.sync.dma_start(out=dbg["osc"][:, :], in_=osc[:, 0:N])
        if fin_stage is not None:
            final_stage(*fin_stage)
        if attn_stage is not None:
            fin_stage = (attn_stage[0], osc, rcol)
        if qk_stage is not None:
            attn_stage = (qk_stage[0], prev_vaug, pexp)
        qk_stage = (pt, qT, kT)
        prev_vaug = vaug

    # drain the ladder
    pexp = scores_stage(*qk_stage[1:])
    osc, rcol = attn_back(*attn_stage[1:])
    final_stage(*fin_stage)
    fin_stage = (attn_stage[0], osc, rcol)
    osc, rcol = attn_back(prev_vaug, pexp)
    final_stage(*fin_stage)
    final_stage(qk_stage[0], osc, rcol)


def build_nc(tok=TOK_FULL, debug_taps=False):
    nc = bacc.Bacc("TRN2", target_bir_lowering=False, debug=False)
    # x fp16, host-blocked partition-major: row = pt*128 + p holds that
    # partition's full 6KB [c, t] slice contiguously
    x_blk = nc.dram_tensor("x_blk", [tok // PT * 128, NC_CHUNKS * PT], F16,
                           kind="ExternalInput").ap()
    w_qkv = nc.dram_tensor("w_qkv", [D, 3 * DH], F16, kind="ExternalInput").ap()
    w_out = nc.dram_tensor("w_out", [DH, D], F16, kind="ExternalInput").ap()
    b_out = nc.dram_tensor("b_out", [D], F16, kind="ExternalInput").ap()
    # y partition-major per ptile (host un-permutes): row pt*128+p = [kc, d]
    y = nc.dram_tensor("y", [tok // PT * 128, 4 * D], F16,
                       kind="ExternalOutput").ap()

    dbg = None
    if debug_taps:
        dbg = {
            "qT": nc.dram_tensor("dbg_qT", [64, PT], F16,
                                 kind="ExternalOutput").ap(),
            "kT": nc.dram_tensor("dbg_kT", [64, PT], F16,
                                 kind="ExternalOutput").ap(),
            "vT": nc.dram_tensor("dbg_vT", [64, PT], F16,
                                 kind="ExternalOutput").ap(),
            "pexp": nc.dram_tensor("dbg_pexp", [128, 2 * W], F16,
                                   kind="ExternalOutput").ap(),
            "osc": nc.dram_tensor("dbg_osc", [65, N], F16,
                                  kind="ExternalOutput").ap(),
            "vaug": nc.dram_tensor("dbg_vaug", [128, 4 * 65], F16,
                                   kind="ExternalOutput").ap(),
        }

    from contextlib import ExitStack
    with tile.TileContext(nc) as tc:
        with ExitStack() as ctx:
            build_body(tc, x_blk, w_qkv, w_out, b_out, y, tok, ctx, dbg=dbg)
    nc.compile()
    return nc


def make_in_maps(x, w_qkv, w_out, b_out):
    in_maps = []
    w_qkv16 = np.asarray(w_qkv, dtype=np.float16)
    w_out16 = np.asarray(w_out, dtype=np.float16)
    b_out16 = np.asarray(b_out, dtype=np.float16)
    npt = TOK_FULL // PT
    for c in range(NCORES):
        xc = np.asarray(x)[c * BLOC:(c + 1) * BLOC].reshape(TOK_FULL, D)
        x16 = xc.astype(np.float16)
        # [tok, D] -> [npt, p, c, t]: per-partition rows are contiguous 6KB
        blk = np.ascontiguousarray(
            x16.reshape(npt, PT, NC_CHUNKS, 128).transpose(0, 3, 2, 1)
        ).reshape(npt * 128, NC_CHUNKS * PT)
        in_maps.append({
            "x_blk": blk,
            "w_qkv": w_qkv16, "w_out": w_out16, "b_out": b_out16,
        })
    return in_maps


_NC_CACHE = {}


def run(x, w_qkv, w_out, b_out, trace=False, **trace_kwargs):
    if "nc" not in _NC_CACHE:
        _NC_CACHE["nc"] = build_nc()
    nc = _NC_CACHE["nc"]
    in_maps = make_in_maps(x, w_qkv, w_out, b_out)
    res = bass_utils.run_bass_kernel_spmd(
        nc, in_maps, core_ids=list(range(NCORES)), trace=trace, **trace_kwargs
    )
    npt = TOK_FULL // PT
    y = np.concatenate(
        [
            res.results[c]["y"]
            .reshape(npt, 128, 4, D)
            .transpose(0, 2, 1, 3)
            .reshape(BLOC, N, D)
            for c in range(NCORES)
        ],
        axis=0,
    )
    return y.astype(np.float32), res


def kernel(x, w_qkv, w_out, b_out):
    y, _ = run(np.asarray(x), np.asarray(w_qkv), np.asarray(w_out),
               np.asarray(b_out))
    return y
